# revision 39
# baseline (speedup 1.0000x reference)
"""Quantum angle-encoder state-vector kernel for Trainium2 (8 NeuronCores).

For each batch row b and qubit q the gate rz*ry applied to |0> contributes a
2-vector col0 = cos(ry/2)e^{-i rz/2}, col1 = sin(ry/2)e^{+i rz/2}; the output
state is the Kronecker product over 16 qubits (qubit 0 = MSB), [B, 2^16] c64.

Per core (32 batch rows, pure data parallel over 8 cores):
  * v = v_hi (x) v_lo with v_hi/v_lo the 8-qubit half-products (length 256),
    built in POLAR form stacked on 64 partitions (rows 0:32 hi, 32:64 lo):
      - phases are additive; they are kept in TURNS (theta/2pi) and summed by
        ONE bf16 matmul against a constant selection matrix.  The fp32 phase
        entries are split h+l into bf16 (16-bit-exact), K=34.  A constant
        ones-column adds +0.25 turns to the cos block, so after the
        round-to-int-and-subtract range reduction one Sin activation with
        scale=2pi yields sin AND cos of all 256 phase sums in one op.
      - magnitudes multiply -> 7-step doubling chain of per-partition-scalar
        broadcasts, first halves on ScalarE, second halves on Pool (small
        steps) / DVE (wide steps).
  * The rel-err budget (2e-2) allows single-bf16 factors (~2.3e-3 end to
    end): the 256x256 outer product is a K=2 bf16 matmul per (b, i-chunk);
    rhs columns are pre-interleaved (lr,li | -li,lr) so PSUM comes out in
    complex64 memory order.
  * 64x [matmul -> PSUM->SBUF copy (3/4 DVE, 1/4 Act) -> SBUF->HBM DMA];
    DMA issues alternate between the SP and Activation HWDGE queues, which
    together sustain ~405 GB/s (the port-0 aggregate wall; the 16.78 MB
    output stream takes ~42.7 us).  A dummy Sin on ScalarE at t0 prefetches
    the activation table off the critical path; staging DMAs are split by
    batch half so early batches' matmuls overlap late staging.

Notes for this toolchain: walrus here encodes at most ONE semaphore wait per
instruction -- _legalize_single_wait() hoists extra Tile-emitted waits into
standalone EventSemaphore instructions. Output per core [32,2,128,512] f32 ==
[32, 65536] complex64 (viewed on host).
"""

import numpy as np

import concourse.bass as bass
import concourse.mybir as mybir
import concourse.tile as tile
from concourse.bass_utils import run_bass_kernel_spmd

N_CORES = 8
B, Q = 256, 16
BC = B // N_CORES  # batch rows per core
HQ = Q // 2  # qubits per half
HL = 1 << HQ  # 256: length of each half-product
F32 = mybir.dt.float32
BF16 = mybir.dt.bfloat16
I32 = mybir.dt.int32
PI = float(np.pi)
PI_HALF = float(np.pi / 2)
TWO_PI = float(2.0 * np.pi)
INV4PI = float(1.0 / (4.0 * np.pi))

_AF = mybir.ActivationFunctionType
_OP = mybir.AluOpType


def _emit_mag_chain(nc, pool, MAGS):
    """Magnitude half of the stacked Kronecker product: per step multiply by
    a per-partition scalar; the two half-writes of each step run on ScalarE
    and Pool in parallel to halve the serial chain latency. [2*BC, HL]."""
    P2 = 2 * BC
    MAG0 = MAGS[:, 0:HQ]
    MAG1 = MAGS[:, HQ : 2 * HQ]
    mA = pool.tile([P2, HL], F32, tag="st_mA")
    mB = pool.tile([P2, HL], F32, tag="st_mB")
    q = HQ - 1
    nc.scalar.copy(mA[:, 0:1], MAG0[:, q : q + 1])
    nc.gpsimd.tensor_copy(mA[:, 1:2], MAG1[:, q : q + 1])
    cur_m, nxt_m = mA, mB
    L = 2
    for q in range(HQ - 2, -1, -1):
        nc.scalar.mul(nxt_m[:, 0:L], cur_m[:, 0:L], MAG0[:, q : q + 1])
        if L <= 16:
            # Pool op cost ~ 170 + 15*L ns: a win only for small steps
            nc.gpsimd.tensor_scalar_mul(
                nxt_m[:, L : 2 * L], cur_m[:, 0:L], MAG1[:, q : q + 1]
            )
        else:
            # wide steps: DVE is ~2x faster than a second ScalarE op
            nc.vector.tensor_scalar_mul(
                nxt_m[:, L : 2 * L], cur_m[:, 0:L], MAG1[:, q : q + 1]
            )
        cur_m, nxt_m = nxt_m, cur_m
        L *= 2
    return cur_m


def _legalize_single_wait(nc):
    """This walrus build encodes at most one semaphore wait per instruction
    ("Too many sync wait commands" otherwise). Hoist extra waits into
    standalone EventSemaphore instructions placed immediately before — a
    sequencer-level wait gates everything after it on the same engine, so
    semantics are preserved (slightly stronger ordering)."""
    cnt = 0
    for fn in nc.m.functions:
        for blk in fn.blocks:
            out = []
            for ins in blk.instructions:
                si = ins.sync_info
                if si is not None and si.on_wait is not None and len(si.on_wait) > 1:
                    waits = list(si.on_wait)
                    for w in waits[:-1]:
                        cnt += 1
                        ev = mybir.InstEventSemaphore(
                            name=f"{ins.name}-presync-{cnt}", ins=[], outs=[]
                        )
                        ev.engine = ins.engine
                        ev.sync_info = mybir.SyncInfo(on_wait=[w], on_update=[])
                        out.append(ev)
                    ins.sync_info = mybir.SyncInfo(
                        on_wait=[waits[-1]], on_update=list(si.on_update)
                    )
                out.append(ins)
            try:
                blk.instructions = out
            except Exception:
                blk.instructions[:] = out
    return cnt


def build_bass():
    nc = bass.Bass()
    ry_d = nc.dram_tensor("ry", [BC, Q], F32, kind="ExternalInput")
    rz_d = nc.dram_tensor("rz", [BC, Q], F32, kind="ExternalInput")
    out_d = nc.dram_tensor("out", [BC, 2, 128, 512], F32, kind="ExternalOutput")

    import ml_dtypes

    P2 = 2 * BC
    ident_np = np.eye(P2, dtype=ml_dtypes.bfloat16)
    ident_d = nc.inline_tensor(ident_np, name="ident_const")

    # Selection matrix for the phase sums, in bf16, used twice (h and l
    # splits of the phase entries).  Row layout matches PHI2 columns:
    # rows 0:8 = phi0 per qubit (h), 8:16 = phi1 (h), 16 = ones-row offsets,
    # 17:33 = same selection for the l split, 33 = zeros.
    sel_np = np.zeros((2 * HQ, HL), dtype=np.float32)
    for q in range(HQ):
        for t in range(2):
            bits = (np.arange(HL) >> (HQ - 1 - q)) & 1
            sel_np[t * HQ + q, :] = (bits == t).astype(np.float32)
    KTH = 2 * (2 * HQ + 1)  # 34
    sel2_np = np.zeros((KTH, 2 * HL), dtype=np.float32)
    sel2_np[0 : 2 * HQ, 0:HL] = sel_np
    sel2_np[0 : 2 * HQ, HL : 2 * HL] = sel_np
    sel2_np[2 * HQ, 0:HL] = 0.0  # sin block: no offset needed (rint reduce)
    sel2_np[2 * HQ, HL : 2 * HL] = 0.25  # cos block: extra quarter turn
    sel2_np[2 * HQ + 1 : 2 * (2 * HQ) + 1, 0:HL] = sel_np
    sel2_np[2 * HQ + 1 : 2 * (2 * HQ) + 1, HL : 2 * HL] = sel_np
    # 8.5 and 8.75 are exact in bf16; selection entries are 0/1: exact.
    sel2_d = nc.inline_tensor(sel2_np.astype(ml_dtypes.bfloat16), name="sel2_const")

    with tile.TileContext(nc) as tc:
        with (
            tc.tile_pool(name="io", bufs=1) as io,
            tc.tile_pool(name="stage", bufs=18) as stage,
            tc.tile_pool(name="psum", bufs=8, space="PSUM") as psum,
        ):
            # act-table prefetch: a dummy Sin as the FIRST scalar instruction
            # pulls the 1.3us ACT_TABLE_LOAD off the critical path
            pih = io.tile([P2, 1], F32, tag="pih")
            nc.gpsimd.memset(pih[:], PI_HALF)
            scr = io.tile([P2, 1], F32, tag="scr")
            nc.scalar.activation(scr[:], pih[:], _AF.Sin)

            sry = io.tile([P2, HQ], F32, tag="sry")
            srz = io.tile([P2, HQ], F32, tag="srz")
            nc.sync.dma_start(sry[0:BC, :], ry_d[:, 0:HQ])
            nc.sync.dma_start(sry[BC:P2, :], ry_d[:, HQ:Q])
            nc.scalar.dma_start(srz[0:BC, :], rz_d[:, 0:HQ])
            nc.scalar.dma_start(srz[BC:P2, :], rz_d[:, HQ:Q])
            ident = io.tile([P2, P2], BF16, tag="ident")
            nc.sync.dma_start(ident[:], ident_d[:])
            sel2 = io.tile([KTH, 2 * HL], BF16, tag="sel2")
            nc.sync.dma_start(sel2[:], sel2_d[:])

            # --- per-qubit columns in polar form (phases in TURNS) ---
            # col0 = cos(ry/2) e^{-i rz/2}: mag |cos|, phase -rz/4pi + 0.5*[c<0]
            # col1 = sin(ry/2) e^{+i rz/2}: mag |sin|, phase +rz/4pi + 0.5*[s<0]
            CS = io.tile([P2, 2 * HQ], F32, tag="CS")
            nc.scalar.activation(CS[:, 0:HQ], sry[:], _AF.Sin, bias=pih[:], scale=0.5)
            nc.scalar.activation(CS[:, HQ : 2 * HQ], sry[:], _AF.Sin, scale=0.5)
            MAGS = io.tile([P2, 2 * HQ], F32, tag="MAGS")
            nc.scalar.activation(MAGS[:], CS[:], _AF.Abs)
            # Sign masks straight from ry (|ry| <= ~4.9 < 2pi for this data):
            # cos(ry/2) < 0 iff |ry| > pi, sin(ry/2) < 0 iff ry < 0 -- the
            # DVE phase path never waits on the ScalarE Sin activations.
            ary = io.tile([P2, HQ], F32, tag="ary")
            nc.vector.tensor_scalar(
                ary[:].bitcast(I32), sry[:].bitcast(I32), 0x7FFFFFFF, None,
                op0=_OP.bitwise_and,
            )
            MK = io.tile([P2, 2 * HQ], F32, tag="MK")
            nc.vector.tensor_scalar(MK[:, 0:HQ], ary[:], PI, None, op0=_OP.is_gt)
            nc.vector.tensor_scalar(MK[:, HQ : 2 * HQ], sry[:], 0.0, None, op0=_OP.is_lt)
            hrz = io.tile([P2, HQ], F32, tag="hrz")
            nc.vector.tensor_scalar_mul(hrz[:], srz[:], INV4PI)

            # PHI [64, 17] f32: cols 0:8 phi0, 8:16 phi1 (turns), col 16 ones
            NPH = 2 * HQ + 1  # 17
            PHI = io.tile([P2, NPH], F32, tag="PHI")
            nc.gpsimd.memset(PHI[:, 2 * HQ : NPH], 1.0)
            nc.vector.scalar_tensor_tensor(
                PHI[:, 0:HQ], MK[:, 0:HQ], 0.5, hrz[:], op0=_OP.mult, op1=_OP.subtract
            )
            nc.vector.scalar_tensor_tensor(
                PHI[:, HQ : 2 * HQ],
                MK[:, HQ : 2 * HQ],
                0.5,
                hrz[:],
                op0=_OP.mult,
                op1=_OP.add,
            )
            # split h+l into bf16 (16-bit-exact phases), free-dim stacked
            PHI2 = io.tile([P2, KTH], BF16, tag="PHI2")
            nc.vector.tensor_copy(PHI2[:, 0:NPH], PHI[:])
            PHIr = io.tile([P2, NPH], F32, tag="PHIr")
            nc.vector.tensor_sub(PHIr[:], PHI[:], PHI2[:, 0:NPH])
            nc.vector.tensor_copy(PHI2[:, NPH:KTH], PHIr[:])

            # one transpose + one K=34 bf16 matmul -> all 256 phase sums,
            # duplicated with +0.25 turns in the cos block
            tp = psum.tile([KTH, P2], BF16, tag="tp", bufs=1)
            nc.tensor.transpose(tp[:], PHI2[:], ident[:])
            vals = io.tile([KTH, P2], BF16, tag="vals")
            nc.vector.tensor_copy(vals[:], tp[:])
            TLM = psum.tile([P2, 2 * HL], F32, tag="tlm", bufs=1)
            nc.tensor.matmul(TLM[:], vals[:], sel2[:], start=True, stop=True)

            # range-reduce (round-to-int cast and subtract), then one Sin
            # activation with scale=2pi yields sin AND cos of all phase sums
            ni = io.tile([P2, 2 * HL], I32, tag="ni")
            nc.vector.tensor_copy(ni[:], TLM[:])
            red = io.tile([P2, 2 * HL], F32, tag="red")
            nc.vector.tensor_sub(red[:], TLM[:], ni[:])
            SC = io.tile([P2, 2 * HL], F32, tag="SC")
            nc.scalar.activation(SC[:], red[:], _AF.Sin, scale=TWO_PI)

            cur_m = _emit_mag_chain(nc, io, MAGS)

            # hi-half factors, bf16, [32, 512]: cols 0:256 = vr, 256:512 = vi
            V = io.tile([BC, 2 * HL], BF16, tag="V")
            nc.vector.tensor_mul(V[:, 0:HL], cur_m[0:BC, :], SC[0:BC, HL : 2 * HL])
            nc.vector.tensor_mul(V[:, HL : 2 * HL], cur_m[0:BC, :], SC[0:BC, 0:HL])
            # lo-half interleaved rhs patterns, bf16, rows 32:64 (partition
            # bases must match the lo-half sources): PT1 = (lr,li) cols 0:512,
            # PT2 = (-li,lr) cols 512:1024
            PTC = io.tile([P2, 4 * HL], BF16, tag="PTC")
            pt = PTC[BC:P2, :]
            v2 = pt[:, 2 * HL : 4 * HL].rearrange("p (j t) -> p j t", t=2)
            nc.vector.scalar_tensor_tensor(
                v2[:, :, 0], cur_m[BC:P2, :], -1.0, SC[BC:P2, 0:HL],
                op0=_OP.mult, op1=_OP.mult,
            )
            nc.vector.tensor_mul(v2[:, :, 1], cur_m[BC:P2, :], SC[BC:P2, HL : 2 * HL])
            v1 = pt[:, 0 : 2 * HL].rearrange("p (j t) -> p j t", t=2)
            nc.vector.tensor_mul(v1[:, :, 0], cur_m[BC:P2, :], SC[BC:P2, HL : 2 * HL])
            nc.vector.tensor_mul(v1[:, :, 1], cur_m[BC:P2, :], SC[BC:P2, 0:HL])

            # staging: flatten batch-major; split by batch half so the first
            # 16 batches' matmuls start while the second half stages
            HB = BC // 2
            LH = io.tile([2, BC * HL], BF16, tag="LH")
            RH = io.tile([2, BC * 2 * HL], BF16, tag="RH")
            nc.sync.dma_start(LH[0:1, 0 : HB * HL], V[0:HB, 0:HL])
            nc.scalar.dma_start(LH[1:2, 0 : HB * HL], V[0:HB, HL : 2 * HL])
            nc.sync.dma_start(RH[1:2, 0 : HB * 2 * HL], pt[0:HB, 2 * HL : 4 * HL])
            nc.scalar.dma_start(RH[0:1, 0 : HB * 2 * HL], pt[0:HB, 0 : 2 * HL])
            nc.sync.dma_start(LH[0:1, HB * HL :], V[HB:BC, 0:HL])
            nc.scalar.dma_start(LH[1:2, HB * HL :], V[HB:BC, HL : 2 * HL])
            nc.sync.dma_start(RH[1:2, HB * 2 * HL :], pt[HB:BC, 2 * HL : 4 * HL])
            nc.scalar.dma_start(RH[0:1, HB * 2 * HL :], pt[HB:BC, 0 : 2 * HL])

            # --- stream: out[b, ck*128+p, :] = hi[b, ck*128+p] * lo[b, :] ---
            for it in range(2 * BC):
                bi, ck = it // 2, it % 2
                acc = psum.tile([128, 512], F32, tag="acc", bufs=5)
                lh_off = bi * HL + ck * 128
                rh_off = bi * 2 * HL
                nc.tensor.matmul(
                    acc[:],
                    LH[:, lh_off : lh_off + 128],
                    RH[:, rh_off : rh_off + 2 * HL],
                    start=True,
                    stop=True,
                )
                ot = stage.tile([128, 512], F32, tag="ot")
                if it % 4 == 3:
                    nc.scalar.copy(ot[:], acc[:])
                else:
                    nc.vector.tensor_copy(ot[:], acc[:])
                out_eng = nc.sync if it % 2 == 0 else nc.scalar
                out_eng.dma_start(out_d[bi, ck], ot[:])
    _legalize_single_wait(nc)
    return nc


_nc_cache = None


def _get_nc():
    global _nc_cache
    if _nc_cache is None:
        _nc_cache = build_bass()
    return _nc_cache


def run(ry_angles, rz_angles, trace=False):
    """Shard over 8 cores, run, gather. Returns (out [B, 2**Q] c64, results)."""
    ry = np.ascontiguousarray(np.asarray(ry_angles, dtype=np.float32))
    rz = np.ascontiguousarray(np.asarray(rz_angles, dtype=np.float32))
    assert ry.shape == (B, Q) and rz.shape == (B, Q)
    nc = _get_nc()
    in_maps = [
        {
            "ry": np.ascontiguousarray(ry[k * BC : (k + 1) * BC]),
            "rz": np.ascontiguousarray(rz[k * BC : (k + 1) * BC]),
        }
        for k in range(N_CORES)
    ]
    res = run_bass_kernel_spmd(nc, in_maps, list(range(N_CORES)), trace=trace)
    parts = [
        np.ascontiguousarray(r["out"]).reshape(BC, 2 * (1 << Q)).view(np.complex64)
        for r in res.results
    ]
    return np.concatenate(parts, axis=0), res


def kernel(ry_angles, rz_angles):
    out, _ = run(ry_angles, rz_angles, trace=False)
    return out


# revision 40
# speedup vs baseline: 1.1193x; 1.1193x over previous
"""Quantum angle-encoder state-vector kernel for Trainium2 (8 NeuronCores).

For each batch row b and qubit q the gate rz*ry applied to |0> contributes a
2-vector col0 = cos(ry/2)e^{-i rz/2}, col1 = sin(ry/2)e^{+i rz/2}; the output
state is the Kronecker product over 16 qubits (qubit 0 = MSB), [B, 2^16] c64.

Per core (32 batch rows, pure data parallel over 8 cores):
  * v = v_hi (x) v_lo with v_hi/v_lo the 8-qubit half-products (length 256),
    built in POLAR form stacked on 64 partitions (rows 0:32 hi, 32:64 lo):
      - phases are additive; they are kept in TURNS (theta/2pi) and summed by
        ONE bf16 matmul against a constant selection matrix.  The fp32 phase
        entries are split h+l into bf16 (16-bit-exact), K=34.  A constant
        ones-column adds +0.25 turns to the cos block, so after the
        round-to-int-and-subtract range reduction one Sin activation with
        scale=2pi yields sin AND cos of all 256 phase sums in one op.
      - magnitudes multiply -> 7-step doubling chain of per-partition-scalar
        broadcasts, first halves on ScalarE, second halves on Pool (small
        steps) / DVE (wide steps).
  * The rel-err budget (2e-2) allows single-bf16 factors (~2.3e-3 end to
    end): the 256x256 outer product is a K=2 bf16 matmul per (b, i-chunk);
    rhs columns are pre-interleaved (lr,li | -li,lr) so PSUM comes out in
    complex64 memory order.
  * 64x [matmul -> PSUM->SBUF copy (3/4 DVE, 1/4 Act) -> SBUF->HBM DMA];
    DMA issues alternate between the SP and Activation HWDGE queues, which
    together sustain ~405 GB/s (the port-0 aggregate wall; the 16.78 MB
    output stream takes ~42.7 us).  A dummy Sin on ScalarE at t0 prefetches
    the activation table off the critical path; staging DMAs are split by
    batch half so early batches' matmuls overlap late staging.

Notes for this toolchain: walrus here encodes at most ONE semaphore wait per
instruction -- _legalize_single_wait() hoists extra Tile-emitted waits into
standalone EventSemaphore instructions. Output per core [32,2,128,512] f32 ==
[32, 65536] complex64 (viewed on host).
"""

import numpy as np

import concourse.bass as bass
import concourse.mybir as mybir
import concourse.tile as tile
from concourse.bass_utils import run_bass_kernel_spmd

N_CORES = 8
B, Q = 256, 16
BC = B // N_CORES  # batch rows per core
HQ = Q // 2  # qubits per half
HL = 1 << HQ  # 256: length of each half-product
F32 = mybir.dt.float32
BF16 = mybir.dt.bfloat16
I32 = mybir.dt.int32
PI = float(np.pi)
PI_HALF = float(np.pi / 2)
TWO_PI = float(2.0 * np.pi)
INV4PI = float(1.0 / (4.0 * np.pi))

_AF = mybir.ActivationFunctionType
_OP = mybir.AluOpType


def _emit_mag_chain(nc, pool, MAGS):
    """Magnitude half of the stacked Kronecker product: per step multiply by
    a per-partition scalar; the two half-writes of each step run on ScalarE
    and Pool in parallel to halve the serial chain latency. [2*BC, HL]."""
    P2 = 2 * BC
    MAG0 = MAGS[:, 0:HQ]
    MAG1 = MAGS[:, HQ : 2 * HQ]
    mA = pool.tile([P2, HL], F32, tag="st_mA")
    mB = pool.tile([P2, HL], F32, tag="st_mB")
    q = HQ - 1
    nc.scalar.copy(mA[:, 0:1], MAG0[:, q : q + 1])
    nc.gpsimd.tensor_copy(mA[:, 1:2], MAG1[:, q : q + 1])
    cur_m, nxt_m = mA, mB
    L = 2
    for q in range(HQ - 2, -1, -1):
        nc.scalar.mul(nxt_m[:, 0:L], cur_m[:, 0:L], MAG0[:, q : q + 1])
        if L <= 16:
            # Pool op cost ~ 170 + 15*L ns: a win only for small steps
            nc.gpsimd.tensor_scalar_mul(
                nxt_m[:, L : 2 * L], cur_m[:, 0:L], MAG1[:, q : q + 1]
            )
        else:
            # wide steps: DVE is ~2x faster than a second ScalarE op
            nc.vector.tensor_scalar_mul(
                nxt_m[:, L : 2 * L], cur_m[:, 0:L], MAG1[:, q : q + 1]
            )
        cur_m, nxt_m = nxt_m, cur_m
        L *= 2
    return cur_m


def _legalize_single_wait(nc):
    """This walrus build encodes at most one semaphore wait per instruction
    ("Too many sync wait commands" otherwise). Hoist extra waits into
    standalone EventSemaphore instructions placed immediately before — a
    sequencer-level wait gates everything after it on the same engine, so
    semantics are preserved (slightly stronger ordering)."""
    cnt = 0
    for fn in nc.m.functions:
        for blk in fn.blocks:
            out = []
            for ins in blk.instructions:
                si = ins.sync_info
                if si is not None and si.on_wait is not None and len(si.on_wait) > 1:
                    waits = list(si.on_wait)
                    for w in waits[:-1]:
                        cnt += 1
                        ev = mybir.InstEventSemaphore(
                            name=f"{ins.name}-presync-{cnt}", ins=[], outs=[]
                        )
                        ev.engine = ins.engine
                        ev.sync_info = mybir.SyncInfo(on_wait=[w], on_update=[])
                        out.append(ev)
                    ins.sync_info = mybir.SyncInfo(
                        on_wait=[waits[-1]], on_update=list(si.on_update)
                    )
                out.append(ins)
            try:
                blk.instructions = out
            except Exception:
                blk.instructions[:] = out
    return cnt


def build_bass():
    nc = bass.Bass()
    ry_d = nc.dram_tensor("ry", [BC, Q], F32, kind="ExternalInput")
    rz_d = nc.dram_tensor("rz", [BC, Q], F32, kind="ExternalInput")
    out_d = nc.dram_tensor("out", [BC, 2, 128, 512], F32, kind="ExternalOutput")

    import ml_dtypes

    P2 = 2 * BC
    ident_np = np.eye(P2, dtype=ml_dtypes.bfloat16)
    ident_d = nc.inline_tensor(ident_np, name="ident_const")

    # Selection matrix for the phase sums, in bf16, used twice (h and l
    # splits of the phase entries).  Row layout matches PHI2 columns:
    # rows 0:8 = phi0 per qubit (h), 8:16 = phi1 (h), 16 = ones-row offsets,
    # 17:33 = same selection for the l split, 33 = zeros.
    sel_np = np.zeros((2 * HQ, HL), dtype=np.float32)
    for q in range(HQ):
        for t in range(2):
            bits = (np.arange(HL) >> (HQ - 1 - q)) & 1
            sel_np[t * HQ + q, :] = (bits == t).astype(np.float32)
    KTH = 2 * (2 * HQ + 1)  # 34
    sel2_np = np.zeros((KTH, 2 * HL), dtype=np.float32)
    sel2_np[0 : 2 * HQ, 0:HL] = sel_np
    sel2_np[0 : 2 * HQ, HL : 2 * HL] = sel_np
    sel2_np[2 * HQ, 0:HL] = 0.0  # sin block: no offset needed (rint reduce)
    sel2_np[2 * HQ, HL : 2 * HL] = 0.25  # cos block: extra quarter turn
    sel2_np[2 * HQ + 1 : 2 * (2 * HQ) + 1, 0:HL] = sel_np
    sel2_np[2 * HQ + 1 : 2 * (2 * HQ) + 1, HL : 2 * HL] = sel_np
    # 8.5 and 8.75 are exact in bf16; selection entries are 0/1: exact.
    sel2_d = nc.inline_tensor(sel2_np.astype(ml_dtypes.bfloat16), name="sel2_const")

    with tile.TileContext(nc) as tc:
        with (
            tc.tile_pool(name="io", bufs=1) as io,
            tc.tile_pool(name="stage", bufs=18) as stage,
            tc.tile_pool(name="psum", bufs=8, space="PSUM") as psum,
        ):
            # act-table prefetch: a dummy Sin as the FIRST scalar instruction
            # pulls the 1.3us ACT_TABLE_LOAD off the critical path
            pih = io.tile([P2, 1], F32, tag="pih")
            nc.gpsimd.memset(pih[:], PI_HALF)
            scr = io.tile([P2, 1], F32, tag="scr")
            nc.scalar.activation(scr[:], pih[:], _AF.Sin)

            sry = io.tile([P2, HQ], F32, tag="sry")
            srz = io.tile([P2, HQ], F32, tag="srz")
            nc.sync.dma_start(sry[0:BC, :], ry_d[:, 0:HQ])
            nc.sync.dma_start(sry[BC:P2, :], ry_d[:, HQ:Q])
            nc.scalar.dma_start(srz[0:BC, :], rz_d[:, 0:HQ])
            nc.scalar.dma_start(srz[BC:P2, :], rz_d[:, HQ:Q])
            ident = io.tile([P2, P2], BF16, tag="ident")
            nc.sync.dma_start(ident[:], ident_d[:])
            sel2 = io.tile([KTH, 2 * HL], BF16, tag="sel2")
            nc.sync.dma_start(sel2[:], sel2_d[:])

            # --- per-qubit columns in polar form (phases in TURNS) ---
            # col0 = cos(ry/2) e^{-i rz/2}: mag |cos|, phase -rz/4pi + 0.5*[c<0]
            # col1 = sin(ry/2) e^{+i rz/2}: mag |sin|, phase +rz/4pi + 0.5*[s<0]
            CS = io.tile([P2, 2 * HQ], F32, tag="CS")
            nc.scalar.activation(CS[:, 0:HQ], sry[:], _AF.Sin, bias=pih[:], scale=0.5)
            nc.scalar.activation(CS[:, HQ : 2 * HQ], sry[:], _AF.Sin, scale=0.5)
            MAGS = io.tile([P2, 2 * HQ], F32, tag="MAGS")
            nc.scalar.activation(MAGS[:], CS[:], _AF.Abs)
            MK = io.tile([P2, 2 * HQ], F32, tag="MK")
            nc.vector.tensor_scalar(MK[:], CS[:], 0.0, None, op0=_OP.is_lt)
            hrz = io.tile([P2, HQ], F32, tag="hrz")
            nc.vector.tensor_scalar_mul(hrz[:], srz[:], INV4PI)

            # PHI [64, 17] f32: cols 0:8 phi0, 8:16 phi1 (turns), col 16 ones
            NPH = 2 * HQ + 1  # 17
            PHI = io.tile([P2, NPH], F32, tag="PHI")
            nc.gpsimd.memset(PHI[:, 2 * HQ : NPH], 1.0)
            nc.vector.scalar_tensor_tensor(
                PHI[:, 0:HQ], MK[:, 0:HQ], 0.5, hrz[:], op0=_OP.mult, op1=_OP.subtract
            )
            nc.vector.scalar_tensor_tensor(
                PHI[:, HQ : 2 * HQ],
                MK[:, HQ : 2 * HQ],
                0.5,
                hrz[:],
                op0=_OP.mult,
                op1=_OP.add,
            )
            # split h+l into bf16 (16-bit-exact phases), free-dim stacked
            PHI2 = io.tile([P2, KTH], BF16, tag="PHI2")
            nc.vector.tensor_copy(PHI2[:, 0:NPH], PHI[:])
            PHIr = io.tile([P2, NPH], F32, tag="PHIr")
            nc.vector.tensor_sub(PHIr[:], PHI[:], PHI2[:, 0:NPH])
            nc.vector.tensor_copy(PHI2[:, NPH:KTH], PHIr[:])

            # one transpose + one K=34 bf16 matmul -> all 256 phase sums,
            # duplicated with +0.25 turns in the cos block
            tp = psum.tile([KTH, P2], BF16, tag="tp", bufs=1)
            nc.tensor.transpose(tp[:], PHI2[:], ident[:])
            vals = io.tile([KTH, P2], BF16, tag="vals")
            nc.vector.tensor_copy(vals[:], tp[:])
            TLM = psum.tile([P2, 2 * HL], F32, tag="tlm", bufs=1)
            nc.tensor.matmul(TLM[:], vals[:], sel2[:], start=True, stop=True)

            # range-reduce (round-to-int cast and subtract), then one Sin
            # activation with scale=2pi yields sin AND cos of all phase sums
            ni = io.tile([P2, 2 * HL], I32, tag="ni")
            nc.vector.tensor_copy(ni[:], TLM[:])
            red = io.tile([P2, 2 * HL], F32, tag="red")
            nc.vector.tensor_sub(red[:], TLM[:], ni[:])
            SC = io.tile([P2, 2 * HL], F32, tag="SC")
            nc.scalar.activation(SC[:], red[:], _AF.Sin, scale=TWO_PI)

            cur_m = _emit_mag_chain(nc, io, MAGS)

            # hi-half factors, bf16, [32, 512]: cols 0:256 = vr, 256:512 = vi
            V = io.tile([BC, 2 * HL], BF16, tag="V")
            nc.vector.tensor_mul(V[:, 0:HL], cur_m[0:BC, :], SC[0:BC, HL : 2 * HL])
            nc.vector.tensor_mul(V[:, HL : 2 * HL], cur_m[0:BC, :], SC[0:BC, 0:HL])
            # lo-half interleaved rhs patterns, bf16, rows 32:64 (partition
            # bases must match the lo-half sources): PT1 = (lr,li) cols 0:512,
            # PT2 = (-li,lr) cols 512:1024
            PTC = io.tile([P2, 4 * HL], BF16, tag="PTC")
            pt = PTC[BC:P2, :]
            v2 = pt[:, 2 * HL : 4 * HL].rearrange("p (j t) -> p j t", t=2)
            nc.vector.scalar_tensor_tensor(
                v2[:, :, 0], cur_m[BC:P2, :], -1.0, SC[BC:P2, 0:HL],
                op0=_OP.mult, op1=_OP.mult,
            )
            nc.vector.tensor_mul(v2[:, :, 1], cur_m[BC:P2, :], SC[BC:P2, HL : 2 * HL])
            v1 = pt[:, 0 : 2 * HL].rearrange("p (j t) -> p j t", t=2)
            nc.vector.tensor_mul(v1[:, :, 0], cur_m[BC:P2, :], SC[BC:P2, HL : 2 * HL])
            nc.vector.tensor_mul(v1[:, :, 1], cur_m[BC:P2, :], SC[BC:P2, 0:HL])

            # staging: flatten batch-major; split by batch half so the first
            # 16 batches' matmuls start while the second half stages
            HB = BC // 2
            LH = io.tile([2, BC * HL], BF16, tag="LH")
            RH = io.tile([2, BC * 2 * HL], BF16, tag="RH")
            nc.sync.dma_start(LH[0:1, 0 : HB * HL], V[0:HB, 0:HL])
            nc.scalar.dma_start(LH[1:2, 0 : HB * HL], V[0:HB, HL : 2 * HL])
            nc.sync.dma_start(RH[1:2, 0 : HB * 2 * HL], pt[0:HB, 2 * HL : 4 * HL])
            nc.scalar.dma_start(RH[0:1, 0 : HB * 2 * HL], pt[0:HB, 0 : 2 * HL])
            nc.sync.dma_start(LH[0:1, HB * HL :], V[HB:BC, 0:HL])
            nc.scalar.dma_start(LH[1:2, HB * HL :], V[HB:BC, HL : 2 * HL])
            nc.sync.dma_start(RH[1:2, HB * 2 * HL :], pt[HB:BC, 2 * HL : 4 * HL])
            nc.scalar.dma_start(RH[0:1, HB * 2 * HL :], pt[HB:BC, 0 : 2 * HL])

            # --- stream: out[b, ck*128+p, :] = hi[b, ck*128+p] * lo[b, :] ---
            for it in range(2 * BC):
                bi, ck = it // 2, it % 2
                acc = psum.tile([128, 512], F32, tag="acc", bufs=5)
                lh_off = bi * HL + ck * 128
                rh_off = bi * 2 * HL
                nc.tensor.matmul(
                    acc[:],
                    LH[:, lh_off : lh_off + 128],
                    RH[:, rh_off : rh_off + 2 * HL],
                    start=True,
                    stop=True,
                )
                ot = stage.tile([128, 512], F32, tag="ot")
                if it % 4 == 3:
                    nc.scalar.copy(ot[:], acc[:])
                else:
                    nc.vector.tensor_copy(ot[:], acc[:])
                out_eng = nc.sync if it % 2 == 0 else nc.scalar
                out_eng.dma_start(out_d[bi, ck], ot[:])
    _legalize_single_wait(nc)
    return nc


_nc_cache = None


def _get_nc():
    global _nc_cache
    if _nc_cache is None:
        _nc_cache = build_bass()
    return _nc_cache


def run(ry_angles, rz_angles, trace=False):
    """Shard over 8 cores, run, gather. Returns (out [B, 2**Q] c64, results)."""
    ry = np.ascontiguousarray(np.asarray(ry_angles, dtype=np.float32))
    rz = np.ascontiguousarray(np.asarray(rz_angles, dtype=np.float32))
    assert ry.shape == (B, Q) and rz.shape == (B, Q)
    nc = _get_nc()
    in_maps = [
        {
            "ry": np.ascontiguousarray(ry[k * BC : (k + 1) * BC]),
            "rz": np.ascontiguousarray(rz[k * BC : (k + 1) * BC]),
        }
        for k in range(N_CORES)
    ]
    res = run_bass_kernel_spmd(nc, in_maps, list(range(N_CORES)), trace=trace)
    parts = [
        np.ascontiguousarray(r["out"]).reshape(BC, 2 * (1 << Q)).view(np.complex64)
        for r in res.results
    ]
    return np.concatenate(parts, axis=0), res


def kernel(ry_angles, rz_angles):
    out, _ = run(ry_angles, rz_angles, trace=False)
    return out
